# revision 26
# baseline (speedup 1.0000x reference)
"""Trainium2 Bass kernel for nn_AttentionLayer (Bahdanau additive attention).

reference:
    W_hi = values @ W_h                      # [B, Te, ATT]
    U_s  = query @ U_a                       # [B, Td, ATT]
    act  = tanh(W_hi[:,None] + U_s[:,:,None])  # [B, Td, Te, ATT]
    scores = act . V_a                       # [B, Td, Te]
    e = softmax(scores, -1)                  # [B, Td, Te]
    c = e @ values                           # [B, Td, D_ENC]
    return (c, e)

Sharding: data-parallel over batch B=8 across the 8 NeuronCores (one batch
element per core); weights replicated. No collectives needed.

Per-core dataflow (Td=128, Te=512, D=512, ATT=256):
  - PE transposes values/query (batched 4-per-PSUM-bank), computes
    W_hiT [ATT, Te] (bf16 operands: fp32 matmuls lower to 2 HW passes) and
    U_sT [ATT, Td] (f32 for bias precision).
  - Main loop over decoder buffers of TB steps: DVE tensor_scalar adds the
    per-partition scalar U_sT[:, t] onto W_hiT chunks (bf16); ScalarE runs
    one big tanh per buffer (the bottleneck: 16.7M tanh/core, 1 elem/cyc/lane);
    PE contracts act tiles with V via m=1 matmuls col-tiled 4-wide into PSUM;
    DVE drains score rows PSUM->SBUF (lagged so the in-order DVE queue never
    blocks); DMA compacts partition-strided rows to scores[Td, Te].
  - Softmax runs per 32-row wave, pipelined inside the main loop; context
    c = (p @ values) * 1/sum via PE (bf16) + DVE scale.
"""

import os
import sys

import numpy as np

_REPO = "/opt/trn_rl_repo"
if _REPO not in sys.path:
    sys.path.insert(0, _REPO)

import concourse.bass as bass  # noqa: E402
import concourse.mybir as mybir  # noqa: E402
import concourse.tile as tile  # noqa: E402
from concourse import bacc  # noqa: E402
from concourse.bass_utils import run_bass_kernel_spmd  # noqa: E402
from concourse.masks import make_identity  # noqa: E402

F32 = mybir.dt.float32
BF16 = mybir.dt.bfloat16
AF = mybir.ActivationFunctionType
ALU = mybir.AluOpType

B, Te, Td, D, ATT = 8, 512, 128, 512, 256
P = 128          # partitions
EC = D // P      # 4 e-chunks
SC = Te // P     # 4 s-chunks
AC = ATT // P    # 2 a-chunks
TB = 8           # decoder steps per tanh buffer
NB = Td // TB    # 16 buffers
WAVE = 32        # softmax wave (rows of scores completed together)
N_CORES = 8


def build_bass() -> bass.Bass:
    nc = bacc.Bacc("TRN2", target_bir_lowering=False, debug=False)

    values_h = nc.declare_dram_parameter("values", [Te, D], F32, isOutput=False)
    query_h = nc.declare_dram_parameter("query", [Td, D], F32, isOutput=False)
    wh_h = nc.declare_dram_parameter("W_h", [D, ATT], F32, isOutput=False)
    ua_h = nc.declare_dram_parameter("U_a", [D, ATT], F32, isOutput=False)
    va_h = nc.declare_dram_parameter("V_a", [1, ATT], F32, isOutput=False)
    c_out_h = nc.declare_dram_parameter("c_out", [Td, D], F32, isOutput=True)
    e_out_h = nc.declare_dram_parameter("e_out", [Td, Te], F32, isOutput=True)

    with tile.TileContext(nc) as tc:
        with (
            tc.tile_pool(name="consts", bufs=1) as consts,
            tc.tile_pool(name="statics", bufs=1) as statics,
            tc.tile_pool(name="pre", bufs=3) as pre_pool,
            tc.tile_pool(name="acts", bufs=3) as act_pool,
            tc.tile_pool(name="scat", bufs=4) as scat_pool,
            tc.tile_pool(name="misc_ps", bufs=2, space="PSUM") as misc_ps,
            tc.tile_pool(name="score_ps", bufs=4, space="PSUM") as score_ps,
        ):
            identity = consts.tile([P, P], F32)
            make_identity(nc, identity)
            identity_bf = consts.tile([P, P], BF16)
            nc.gpsimd.tensor_copy(out=identity_bf, in_=identity)

            # ---------------- load inputs (values first: longest pole) -------
            # values arrives in COLUMN chunks so the transpose -> W_hiT
            # pipeline can run per-e-chunk as data lands.
            values_sb = statics.tile([P, SC, D], F32)    # [s-part, s-chunk, e]
            values_r = values_h[:].rearrange("(c p) e -> p c e", p=P)
            wh_sb = statics.tile([P, EC, ATT], F32)      # [e-part, e-chunk, a]
            nc.scalar.dma_start(
                out=wh_sb, in_=wh_h[:].rearrange("(c p) a -> p c a", p=P)
            )
            for ec in range(EC):
                # issue from ScalarE's DGE: parallel with Sync's issues below
                nc.scalar.dma_start(
                    out=values_sb[:, :, ec * P:(ec + 1) * P],
                    in_=values_r[:, :, ec * P:(ec + 1) * P],
                )
            query_sb = statics.tile([P, D], F32)         # [t, e2]
            nc.sync.dma_start(out=query_sb, in_=query_h[:])
            ua_sb = statics.tile([P, EC, ATT], F32)
            nc.sync.dma_start(
                out=ua_sb, in_=ua_h[:].rearrange("(c p) a -> p c a", p=P)
            )
            v_sb = statics.tile([P, AC], F32)            # V_a chunks on partitions
            nc.sync.dma_start(
                out=v_sb, in_=va_h[:].rearrange("o (c p) -> p (o c)", p=P)
            )
            v_bf = statics.tile([P, AC], BF16)
            nc.vector.tensor_copy(out=v_bf, in_=v_sb)
            wh_bf = statics.tile([P, EC, ATT], BF16)
            nc.gpsimd.tensor_copy(out=wh_bf, in_=wh_sb)

            # ------- valuesT + W_hiT pipelined per e-chunk as DMA lands ------
            # valt_bf[:, ec, :] = values[:, ec-chunk].T  -> [e-part, e-chunk, s]
            # wh_ps[:, ai, :] += W_h[ec, ai-chunk].T @ valt[ec]   (bf16 matmul)
            values_bf = statics.tile([P, SC, D], BF16)
            valt_bf = statics.tile([P, EC, Te], BF16)
            whT_bf = statics.tile([P, AC, Te], BF16)
            wh_ps = misc_ps.tile([P, AC, Te], F32, tag="ps_wh", bufs=1)
            for ec in range(EC):
                blk = slice(ec * P, (ec + 1) * P)
                nc.vector.tensor_copy(
                    out=values_bf[:, :, blk], in_=values_sb[:, :, blk]
                )
                tp_ps = misc_ps.tile([P, SC, P], BF16, tag="ps_misc")
                for sc in range(SC):
                    nc.tensor.transpose(
                        tp_ps[:, sc, :],
                        values_bf[:, sc, blk],
                        identity_bf,
                    )
                nc.vector.tensor_copy(out=valt_bf[:, ec, :], in_=tp_ps)
                for ai in range(AC):
                    nc.tensor.matmul(
                        wh_ps[:, ai, :],
                        wh_bf[:, ec, ai * P:(ai + 1) * P],   # [e-chunk, a-chunk]
                        valt_bf[:, ec, :],                   # [e-chunk, s]
                        start=(ec == 0),
                        stop=(ec == EC - 1),
                    )
            for ai in range(AC):
                nc.scalar.copy(out=whT_bf[:, ai, :], in_=wh_ps[:, ai, :])

            # casts needed a bit later; behind the values casts on DVE
            ua_bf = statics.tile([P, EC, ATT], BF16)
            nc.vector.tensor_copy(out=ua_bf, in_=ua_sb)

            # ---------------- queryT via PE transpose ------------------------
            qT_bf = statics.tile([P, EC, Td], BF16)      # [e2-part, e2-chunk, t]
            tq_ps = misc_ps.tile([P, EC, P], F32, tag="ps_misc")
            for qc in range(EC):
                nc.tensor.transpose(
                    tq_ps[:, qc, :], query_sb[:, qc * P:(qc + 1) * P], identity
                )
            nc.scalar.copy(out=qT_bf, in_=tq_ps)

            # ---------------- U_sT = (query @ U_a).T  [a, t] -----------------
            usT_sb = statics.tile([P, AC, Td], F32)
            us_ps = misc_ps.tile([P, AC, Td], F32, tag="ps_misc")
            for ai in range(AC):
                for qc in range(EC):
                    nc.tensor.matmul(
                        us_ps[:, ai, :],
                        ua_bf[:, qc, ai * P:(ai + 1) * P],
                        qT_bf[:, qc, :],
                        start=(qc == 0),
                        stop=(qc == EC - 1),
                    )
            nc.scalar.copy(out=usT_sb, in_=us_ps)

            # ---------------- main loop: tanh cube + V reduction -------------
            # Score drains are emitted LAG groups behind their producers so the
            # in-order DVE queue never blocks on a drain whose PE/ACT inputs
            # aren't ready (that would stall the next buffer's pre-adds).
            scores_sb = statics.tile([P, Te], F32)       # [t, s]
            p_sb = statics.tile([P, Te], F32)            # exp(scores - max)
            p_bf = statics.tile([P, Te], BF16)
            negmax = statics.tile([P, 1], F32)
            ssum = statics.tile([P, 1], F32)
            rsum = statics.tile([P, 1], F32)
            e_sb = statics.tile([P, Te], F32)

            LAG = 2
            pending = []
            drained = 0

            def emit_drain():
                nonlocal drained
                sc_ps, g = pending.pop(0)
                scat = scat_pool.tile([P, Te], F32, tag="scat")
                nc.vector.tensor_copy(out=scat, in_=sc_ps)
                # compact partitions {0,32,64,96} -> scores rows 4g..4g+3
                scat_strided = scat[:].rearrange(
                    "(j r) f -> j r f", r=32
                )[:, 0, :]
                nc.sync.dma_start(
                    out=scores_sb[4 * g:4 * g + 4, :], in_=scat_strided
                )
                drained += 1
                if drained % (WAVE // 4) == 0:
                    emit_softmax_wave(drained // (WAVE // 4) - 1)

            def emit_softmax_wave(w):
                # rows 32w..32w+32 of scores are compacted; softmax them.
                rows = slice(WAVE * w, WAVE * (w + 1))
                nc.vector.tensor_reduce(
                    out=negmax[rows], in_=scores_sb[rows, :],
                    axis=mybir.AxisListType.X, op=ALU.max, negate=True,
                )
                nc.scalar.activation(
                    out=p_sb[rows, :], in_=scores_sb[rows, :], func=AF.Exp,
                    bias=negmax[rows], scale=1.0,
                )
                nc.vector.reduce_sum(
                    out=ssum[rows], in_=p_sb[rows, :],
                    axis=mybir.AxisListType.X,
                )
                nc.vector.reciprocal(out=rsum[rows], in_=ssum[rows])
                nc.vector.tensor_scalar_mul(
                    e_sb[rows, :], in0=p_sb[rows, :], scalar1=rsum[rows]
                )
                nc.sync.dma_start(out=e_out_h[rows, :], in_=e_sb[rows, :])
                # bf16 copy of p for the tail's transposes + context matmul
                nc.vector.tensor_copy(out=p_bf[rows, :], in_=p_sb[rows, :])

            if os.environ.get("RECIP_TEST"):
                rt_in = statics.tile([P, 2, Te], F32)
                rt_out = statics.tile([P, 2, Te], F32)
                nc.vector.tensor_scalar(
                    out=rt_in, in0=whT_bf, scalar1=1.5, scalar2=None,
                    op0=ALU.add,
                )
                for _ in range(2):
                    nc.vector.reciprocal(out=rt_out, in_=rt_in)

            # First buffers are small so the first tanh starts earlier; the
            # last is small so the post-loop chain starts on a short tanh.
            sizes = [2, 2, 4] + [TB] * ((Td - 16) // TB) + [4, 4]
            assert sum(sizes) == Td
            slots = {}                               # t -> (act_tile, j)
            t0 = 0
            for tbn in sizes:
                pre = pre_pool.tile([P, TB, AC, Te], BF16, tag="pre")
                for ai in range(AC):
                    for j in range(tbn):
                        t = t0 + j
                        # pre[a, s] = W_hiT[a, s] + U_sT[a, t]
                        nc.vector.tensor_scalar(
                            out=pre[:, j, ai, :],
                            in0=whT_bf[:, ai, :],
                            scalar1=usT_sb[:, ai, t:t + 1],
                            scalar2=None,
                            op0=ALU.add,
                        )
                act = act_pool.tile([P, TB, AC, Te], BF16, tag="act")
                nc.scalar.activation(
                    out=act[:, :tbn], in_=pre[:, :tbn], func=AF.Tanh
                )
                for j in range(tbn):
                    slots[t0 + j] = (act, j)
                t0 += tbn

                while (4 * len(pending) + 4 * drained) + 4 <= t0:
                    g = len(pending) + drained       # next group to emit
                    sc_ps = score_ps.tile([P, Te], F32, tag="score")
                    for j2 in range(4):
                        a_tile, jj = slots.pop(4 * g + j2)
                        for ai in range(AC):
                            # score[t, :] += V[a-chunk] . act[a-chunk, :]
                            nc.tensor.matmul(
                                sc_ps[32 * j2:32 * j2 + 1, :],
                                v_bf[:, ai:ai + 1],
                                a_tile[:, jj, ai, :],
                                start=(ai == 0),
                                stop=(ai == AC - 1),
                                tile_position=(0, 32 * j2),
                            )
                    pending.append((sc_ps, g))
                    if len(pending) > LAG:
                        emit_drain()
            while pending:
                emit_drain()

            # ---------------- c = (p @ values) * rsum ------------------------
            pT_bf = statics.tile([P, SC, Td], BF16)      # [s-part, s-chunk, t]
            pt_ps = misc_ps.tile([P, SC, P], BF16, tag="ps_misc")
            for sc in range(SC):
                nc.tensor.transpose(
                    pt_ps[:, sc, :], p_bf[:, sc * P:(sc + 1) * P], identity_bf
                )
                nc.vector.tensor_copy(out=pT_bf[:, sc, :], in_=pt_ps[:, sc, :])

            c_ps = misc_ps.tile([P, D], F32, tag="ps_misc")
            for sc in range(SC):
                nc.tensor.matmul(
                    c_ps,
                    pT_bf[:, sc, :],                     # [s-chunk, t]
                    values_bf[:, sc, :],                 # [s-chunk, e]
                    start=(sc == 0),
                    stop=(sc == SC - 1),
                )
            c_sb = statics.tile([P, D], F32)
            nc.vector.tensor_scalar_mul(c_sb, in0=c_ps, scalar1=rsum)
            nc.sync.dma_start(out=c_out_h[:], in_=c_sb)

    nc.compile()
    return nc


_NC_CACHE = None


def _get_nc():
    global _NC_CACHE
    if _NC_CACHE is None:
        _NC_CACHE = build_bass()
    return _NC_CACHE


def run(inputs: dict, trace: bool = False, **kw):
    """Run the SPMD kernel on 8 cores. Returns (BassKernelResults, c, e)."""
    values = np.asarray(inputs["values"], dtype=np.float32)
    query = np.asarray(inputs["query"], dtype=np.float32)
    w_h = np.ascontiguousarray(np.asarray(inputs["W_h"], dtype=np.float32))
    u_a = np.ascontiguousarray(np.asarray(inputs["U_a"], dtype=np.float32))
    v_a = np.ascontiguousarray(np.asarray(inputs["V_a"], dtype=np.float32))

    in_maps = [
        {
            "values": np.ascontiguousarray(values[i]),
            "query": np.ascontiguousarray(query[i]),
            "W_h": w_h,
            "U_a": u_a,
            "V_a": v_a,
        }
        for i in range(N_CORES)
    ]
    res = run_bass_kernel_spmd(
        _get_nc(), in_maps, list(range(N_CORES)), trace=trace, **kw
    )
    c = np.stack([res.results[i]["c_out"] for i in range(N_CORES)])
    e = np.stack([res.results[i]["e_out"] for i in range(N_CORES)])
    return res, c, e


def kernel(**inputs) -> tuple:
    _, c, e = run(inputs)
    return c, e


if __name__ == "__main__":
    rng = np.random.default_rng(0)
    ins = {
        "values": rng.standard_normal((B, Te, D), dtype=np.float32),
        "query": rng.standard_normal((B, Td, D), dtype=np.float32),
        "W_h": rng.uniform(-0.05, 0.05, (D, ATT)).astype(np.float32),
        "U_a": rng.uniform(-0.05, 0.05, (D, ATT)).astype(np.float32),
        "V_a": rng.uniform(-0.05, 0.05, (1, ATT)).astype(np.float32),
    }
    c, e = kernel(**ins)
    print("c", c.shape, c.dtype, "e", e.shape, e.dtype)


# revision 28
# speedup vs baseline: 1.0135x; 1.0135x over previous
"""Trainium2 Bass kernel for nn_AttentionLayer (Bahdanau additive attention).

reference:
    W_hi = values @ W_h                      # [B, Te, ATT]
    U_s  = query @ U_a                       # [B, Td, ATT]
    act  = tanh(W_hi[:,None] + U_s[:,:,None])  # [B, Td, Te, ATT]
    scores = act . V_a                       # [B, Td, Te]
    e = softmax(scores, -1)                  # [B, Td, Te]
    c = e @ values                           # [B, Td, D_ENC]
    return (c, e)

Sharding: data-parallel over batch B=8 across the 8 NeuronCores (one batch
element per core); weights replicated. No collectives needed.

Per-core dataflow (Td=128, Te=512, D=512, ATT=256):
  - PE transposes values/query (batched 4-per-PSUM-bank), computes
    W_hiT [ATT, Te] (bf16 operands: fp32 matmuls lower to 2 HW passes) and
    U_sT [ATT, Td] (f32 for bias precision).
  - Main loop over decoder buffers of TB steps: DVE tensor_scalar adds the
    per-partition scalar U_sT[:, t] onto W_hiT chunks (bf16); ScalarE runs
    one big tanh per buffer (the bottleneck: 16.7M tanh/core, 1 elem/cyc/lane);
    PE contracts act tiles with V via m=1 matmuls col-tiled 4-wide into PSUM;
    DVE drains score rows PSUM->SBUF (lagged so the in-order DVE queue never
    blocks); DMA compacts partition-strided rows to scores[Td, Te].
  - Softmax runs per 32-row wave, pipelined inside the main loop; context
    c = (p @ values) * 1/sum via PE (bf16) + DVE scale.
"""

import os
import sys

import numpy as np

_REPO = "/opt/trn_rl_repo"
if _REPO not in sys.path:
    sys.path.insert(0, _REPO)

import concourse.bass as bass  # noqa: E402
import concourse.mybir as mybir  # noqa: E402
import concourse.tile as tile  # noqa: E402
from concourse import bacc  # noqa: E402
from concourse.bass_utils import run_bass_kernel_spmd  # noqa: E402
from concourse.masks import make_identity  # noqa: E402

F32 = mybir.dt.float32
BF16 = mybir.dt.bfloat16
AF = mybir.ActivationFunctionType
ALU = mybir.AluOpType

B, Te, Td, D, ATT = 8, 512, 128, 512, 256
P = 128          # partitions
EC = D // P      # 4 e-chunks
SC = Te // P     # 4 s-chunks
AC = ATT // P    # 2 a-chunks
TB = 8           # decoder steps per tanh buffer
NB = Td // TB    # 16 buffers
WAVE = 32        # softmax wave (rows of scores completed together)
N_CORES = 8


def build_bass() -> bass.Bass:
    nc = bacc.Bacc("TRN2", target_bir_lowering=False, debug=False)

    values_h = nc.declare_dram_parameter("values", [Te, D], F32, isOutput=False)
    query_h = nc.declare_dram_parameter("query", [Td, D], F32, isOutput=False)
    wh_h = nc.declare_dram_parameter("W_h", [D, ATT], F32, isOutput=False)
    ua_h = nc.declare_dram_parameter("U_a", [D, ATT], F32, isOutput=False)
    va_h = nc.declare_dram_parameter("V_a", [1, ATT], F32, isOutput=False)
    c_out_h = nc.declare_dram_parameter("c_out", [Td, D], F32, isOutput=True)
    e_out_h = nc.declare_dram_parameter("e_out", [Td, Te], F32, isOutput=True)

    with tile.TileContext(nc) as tc:
        with (
            tc.tile_pool(name="consts", bufs=1) as consts,
            tc.tile_pool(name="statics", bufs=1) as statics,
            tc.tile_pool(name="pre", bufs=3) as pre_pool,
            tc.tile_pool(name="acts", bufs=3) as act_pool,
            tc.tile_pool(name="scat", bufs=4) as scat_pool,
            tc.tile_pool(name="misc_ps", bufs=2, space="PSUM") as misc_ps,
            tc.tile_pool(name="score_ps", bufs=4, space="PSUM") as score_ps,
        ):
            identity = consts.tile([P, P], F32)
            make_identity(nc, identity)
            identity_bf = consts.tile([P, P], BF16)
            nc.gpsimd.tensor_copy(out=identity_bf, in_=identity)

            # ---------------- load inputs (values first: longest pole) -------
            # values arrives in COLUMN chunks so the transpose -> W_hiT
            # pipeline can run per-e-chunk as data lands.
            values_sb = statics.tile([P, SC, D], F32)    # [s-part, s-chunk, e]
            values_r = values_h[:].rearrange("(c p) e -> p c e", p=P)
            for sc in range(SC):
                # issue from ScalarE's DGE: parallel with Sync's issues below
                nc.scalar.dma_start(
                    out=values_sb[:, sc, :], in_=values_r[:, sc, :]
                )
            wh_sb = statics.tile([P, EC, ATT], F32)      # [e-part, e-chunk, a]
            nc.scalar.dma_start(
                out=wh_sb, in_=wh_h[:].rearrange("(c p) a -> p c a", p=P)
            )
            query_sb = statics.tile([P, D], F32)         # [t, e2]
            nc.sync.dma_start(out=query_sb, in_=query_h[:])
            ua_sb = statics.tile([P, EC, ATT], F32)
            nc.sync.dma_start(
                out=ua_sb, in_=ua_h[:].rearrange("(c p) a -> p c a", p=P)
            )
            v_sb = statics.tile([P, AC], F32)            # V_a chunks on partitions
            nc.sync.dma_start(
                out=v_sb, in_=va_h[:].rearrange("o (c p) -> p (o c)", p=P)
            )
            v_bf = statics.tile([P, AC], BF16)
            nc.vector.tensor_copy(out=v_bf, in_=v_sb)

            # ---------------- valuesT via PE transpose (bf16) ----------------
            # valt_bf[:, ec, :] = values[:, ec-chunk].T  -> [e-part, e-chunk, s]
            # Batched per s-chunk so each batch starts as soon as that values
            # chunk's cast lands; one PSUM-bank drain per batch.
            values_bf = statics.tile([P, SC, D], BF16)
            for sc in range(SC):
                nc.vector.tensor_copy(
                    out=values_bf[:, sc, :], in_=values_sb[:, sc, :]
                )
            valt_bf = statics.tile([P, EC, Te], BF16)
            for sc in range(SC):
                tp_ps = misc_ps.tile([P, EC, P], BF16, tag="ps_misc")
                for ec in range(EC):
                    nc.tensor.transpose(
                        tp_ps[:, ec, :],
                        values_bf[:, sc, ec * P:(ec + 1) * P],
                        identity_bf,
                    )
                nc.vector.tensor_copy(
                    out=valt_bf[:, :, sc * P:(sc + 1) * P], in_=tp_ps
                )

            # casts needed a bit later; behind the valuesT drains on DVE
            wh_bf = statics.tile([P, EC, ATT], BF16)
            nc.vector.tensor_copy(out=wh_bf, in_=wh_sb)
            ua_bf = statics.tile([P, EC, ATT], BF16)
            nc.vector.tensor_copy(out=ua_bf, in_=ua_sb)

            # ---------------- W_hiT = (values @ W_h).T  [a, s]  (bf16) -------
            whT_bf = statics.tile([P, AC, Te], BF16)
            wh_ps = misc_ps.tile([P, AC, Te], F32, tag="ps_wh", bufs=1)
            for ec in range(EC):
                for ai in range(AC):
                    nc.tensor.matmul(
                        wh_ps[:, ai, :],
                        wh_bf[:, ec, ai * P:(ai + 1) * P],   # [e-chunk, a-chunk]
                        valt_bf[:, ec, :],                   # [e-chunk, s]
                        start=(ec == 0),
                        stop=(ec == EC - 1),
                    )
            for ai in range(AC):
                nc.scalar.copy(out=whT_bf[:, ai, :], in_=wh_ps[:, ai, :])

            # ---------------- queryT via PE transpose ------------------------
            qT_bf = statics.tile([P, EC, Td], BF16)      # [e2-part, e2-chunk, t]
            tq_ps = misc_ps.tile([P, EC, P], F32, tag="ps_misc")
            for qc in range(EC):
                nc.tensor.transpose(
                    tq_ps[:, qc, :], query_sb[:, qc * P:(qc + 1) * P], identity
                )
            nc.scalar.copy(out=qT_bf, in_=tq_ps)

            # ---------------- U_sT = (query @ U_a).T  [a, t] -----------------
            usT_sb = statics.tile([P, AC, Td], F32)
            us_ps = misc_ps.tile([P, AC, Td], F32, tag="ps_misc")
            for ai in range(AC):
                for qc in range(EC):
                    nc.tensor.matmul(
                        us_ps[:, ai, :],
                        ua_bf[:, qc, ai * P:(ai + 1) * P],
                        qT_bf[:, qc, :],
                        start=(qc == 0),
                        stop=(qc == EC - 1),
                    )
            nc.scalar.copy(out=usT_sb, in_=us_ps)

            # ---------------- main loop: tanh cube + V reduction -------------
            # Score drains are emitted LAG groups behind their producers so the
            # in-order DVE queue never blocks on a drain whose PE/ACT inputs
            # aren't ready (that would stall the next buffer's pre-adds).
            scores_sb = statics.tile([P, Te], F32)       # [t, s]
            p_sb = statics.tile([P, Te], F32)            # exp(scores - max)
            p_bf = statics.tile([P, Te], BF16)
            negmax = statics.tile([P, 1], F32)
            ssum = statics.tile([P, 1], F32)
            rsum = statics.tile([P, 1], F32)
            e_sb = statics.tile([P, Te], F32)

            LAG = 2
            pending = []
            drained = 0

            def emit_drain():
                nonlocal drained
                sc_ps, g = pending.pop(0)
                scat = scat_pool.tile([P, Te], F32, tag="scat")
                nc.vector.tensor_copy(out=scat, in_=sc_ps)
                # compact partitions {0,32,64,96} -> scores rows 4g..4g+3
                scat_strided = scat[:].rearrange(
                    "(j r) f -> j r f", r=32
                )[:, 0, :]
                nc.sync.dma_start(
                    out=scores_sb[4 * g:4 * g + 4, :], in_=scat_strided
                )
                drained += 1
                if drained % (WAVE // 4) == 0:
                    emit_softmax_wave(drained // (WAVE // 4) - 1)

            def emit_softmax_wave(w):
                # rows 32w..32w+32 of scores are compacted; softmax them.
                rows = slice(WAVE * w, WAVE * (w + 1))
                nc.vector.tensor_reduce(
                    out=negmax[rows], in_=scores_sb[rows, :],
                    axis=mybir.AxisListType.X, op=ALU.max, negate=True,
                )
                nc.scalar.activation(
                    out=p_sb[rows, :], in_=scores_sb[rows, :], func=AF.Exp,
                    bias=negmax[rows], scale=1.0,
                )
                nc.vector.reduce_sum(
                    out=ssum[rows], in_=p_sb[rows, :],
                    axis=mybir.AxisListType.X,
                )
                nc.vector.reciprocal(out=rsum[rows], in_=ssum[rows])
                nc.vector.tensor_scalar_mul(
                    e_sb[rows, :], in0=p_sb[rows, :], scalar1=rsum[rows]
                )
                nc.sync.dma_start(out=e_out_h[rows, :], in_=e_sb[rows, :])
                # bf16 copy of p for the tail's transposes + context matmul
                nc.vector.tensor_copy(out=p_bf[rows, :], in_=p_sb[rows, :])

            if os.environ.get("RECIP_TEST"):
                rt_in = statics.tile([P, 2, Te], F32)
                rt_out = statics.tile([P, 2, Te], F32)
                nc.vector.tensor_scalar(
                    out=rt_in, in0=whT_bf, scalar1=1.5, scalar2=None,
                    op0=ALU.add,
                )
                for _ in range(2):
                    nc.vector.reciprocal(out=rt_out, in_=rt_in)

            # First buffers are small so the first tanh starts earlier; the
            # last is small so the post-loop chain starts on a short tanh.
            sizes = [2, 2, 4] + [TB] * ((Td - 16) // TB) + [4, 4]
            assert sum(sizes) == Td
            slots = {}                               # t -> (act_tile, j)
            t0 = 0
            for tbn in sizes:
                pre = pre_pool.tile([P, TB, AC, Te], BF16, tag="pre")
                for ai in range(AC):
                    for j in range(tbn):
                        t = t0 + j
                        # pre[a, s] = W_hiT[a, s] + U_sT[a, t]
                        nc.vector.tensor_scalar(
                            out=pre[:, j, ai, :],
                            in0=whT_bf[:, ai, :],
                            scalar1=usT_sb[:, ai, t:t + 1],
                            scalar2=None,
                            op0=ALU.add,
                        )
                act = act_pool.tile([P, TB, AC, Te], BF16, tag="act")
                nc.scalar.activation(
                    out=act[:, :tbn], in_=pre[:, :tbn], func=AF.Tanh
                )
                for j in range(tbn):
                    slots[t0 + j] = (act, j)
                t0 += tbn

                while (4 * len(pending) + 4 * drained) + 4 <= t0:
                    g = len(pending) + drained       # next group to emit
                    sc_ps = score_ps.tile([P, Te], F32, tag="score")
                    for j2 in range(4):
                        a_tile, jj = slots.pop(4 * g + j2)
                        for ai in range(AC):
                            # score[t, :] += V[a-chunk] . act[a-chunk, :]
                            nc.tensor.matmul(
                                sc_ps[32 * j2:32 * j2 + 1, :],
                                v_bf[:, ai:ai + 1],
                                a_tile[:, jj, ai, :],
                                start=(ai == 0),
                                stop=(ai == AC - 1),
                                tile_position=(0, 32 * j2),
                            )
                    pending.append((sc_ps, g))
                    if len(pending) > LAG:
                        emit_drain()
            while pending:
                emit_drain()

            # ---------------- c = (p @ values) * rsum ------------------------
            pT_bf = statics.tile([P, SC, Td], BF16)      # [s-part, s-chunk, t]
            pt_ps = misc_ps.tile([P, SC, P], BF16, tag="ps_misc")
            for sc in range(SC):
                nc.tensor.transpose(
                    pt_ps[:, sc, :], p_bf[:, sc * P:(sc + 1) * P], identity_bf
                )
                nc.vector.tensor_copy(out=pT_bf[:, sc, :], in_=pt_ps[:, sc, :])

            c_ps = misc_ps.tile([P, D], F32, tag="ps_misc")
            for sc in range(SC):
                nc.tensor.matmul(
                    c_ps,
                    pT_bf[:, sc, :],                     # [s-chunk, t]
                    values_bf[:, sc, :],                 # [s-chunk, e]
                    start=(sc == 0),
                    stop=(sc == SC - 1),
                )
            c_sb = statics.tile([P, D], F32)
            nc.vector.tensor_scalar_mul(c_sb, in0=c_ps, scalar1=rsum)
            nc.sync.dma_start(out=c_out_h[:], in_=c_sb)

    nc.compile()
    return nc


_NC_CACHE = None


def _get_nc():
    global _NC_CACHE
    if _NC_CACHE is None:
        _NC_CACHE = build_bass()
    return _NC_CACHE


def run(inputs: dict, trace: bool = False, **kw):
    """Run the SPMD kernel on 8 cores. Returns (BassKernelResults, c, e)."""
    values = np.asarray(inputs["values"], dtype=np.float32)
    query = np.asarray(inputs["query"], dtype=np.float32)
    w_h = np.ascontiguousarray(np.asarray(inputs["W_h"], dtype=np.float32))
    u_a = np.ascontiguousarray(np.asarray(inputs["U_a"], dtype=np.float32))
    v_a = np.ascontiguousarray(np.asarray(inputs["V_a"], dtype=np.float32))

    in_maps = [
        {
            "values": np.ascontiguousarray(values[i]),
            "query": np.ascontiguousarray(query[i]),
            "W_h": w_h,
            "U_a": u_a,
            "V_a": v_a,
        }
        for i in range(N_CORES)
    ]
    res = run_bass_kernel_spmd(
        _get_nc(), in_maps, list(range(N_CORES)), trace=trace, **kw
    )
    c = np.stack([res.results[i]["c_out"] for i in range(N_CORES)])
    e = np.stack([res.results[i]["e_out"] for i in range(N_CORES)])
    return res, c, e


def kernel(**inputs) -> tuple:
    _, c, e = run(inputs)
    return c, e


if __name__ == "__main__":
    rng = np.random.default_rng(0)
    ins = {
        "values": rng.standard_normal((B, Te, D), dtype=np.float32),
        "query": rng.standard_normal((B, Td, D), dtype=np.float32),
        "W_h": rng.uniform(-0.05, 0.05, (D, ATT)).astype(np.float32),
        "U_a": rng.uniform(-0.05, 0.05, (D, ATT)).astype(np.float32),
        "V_a": rng.uniform(-0.05, 0.05, (1, ATT)).astype(np.float32),
    }
    c, e = kernel(**ins)
    print("c", c.shape, c.dtype, "e", e.shape, e.dtype)


# revision 32
# speedup vs baseline: 1.0205x; 1.0069x over previous
"""Trainium2 Bass kernel for nn_AttentionLayer (Bahdanau additive attention).

reference:
    W_hi = values @ W_h                      # [B, Te, ATT]
    U_s  = query @ U_a                       # [B, Td, ATT]
    act  = tanh(W_hi[:,None] + U_s[:,:,None])  # [B, Td, Te, ATT]
    scores = act . V_a                       # [B, Td, Te]
    e = softmax(scores, -1)                  # [B, Td, Te]
    c = e @ values                           # [B, Td, D_ENC]
    return (c, e)

Sharding: data-parallel over batch B=8 across the 8 NeuronCores (one batch
element per core); weights replicated. No collectives needed.

Per-core dataflow (Td=128, Te=512, D=512, ATT=256):
  - PE transposes values/query (batched 4-per-PSUM-bank), computes
    W_hiT [ATT, Te] (bf16 operands: fp32 matmuls lower to 2 HW passes) and
    U_sT [ATT, Td] (f32 for bias precision).
  - Main loop over decoder buffers of TB steps: DVE tensor_scalar adds the
    per-partition scalar U_sT[:, t] onto W_hiT chunks (bf16); ScalarE runs
    one big tanh per buffer (the bottleneck: 16.7M tanh/core, 1 elem/cyc/lane);
    PE contracts act tiles with V via m=1 matmuls col-tiled 4-wide into PSUM;
    DVE drains score rows PSUM->SBUF (lagged so the in-order DVE queue never
    blocks); DMA compacts partition-strided rows to scores[Td, Te].
  - Softmax runs per 32-row wave, pipelined inside the main loop; context
    c = (p @ values) * 1/sum via PE (bf16) + DVE scale.
"""

import os
import sys

import numpy as np

_REPO = "/opt/trn_rl_repo"
if _REPO not in sys.path:
    sys.path.insert(0, _REPO)

import concourse.bass as bass  # noqa: E402
import concourse.mybir as mybir  # noqa: E402
import concourse.tile as tile  # noqa: E402
from concourse import bacc  # noqa: E402
from concourse.bass_utils import run_bass_kernel_spmd  # noqa: E402
from concourse.masks import make_identity  # noqa: E402

F32 = mybir.dt.float32
BF16 = mybir.dt.bfloat16
AF = mybir.ActivationFunctionType
ALU = mybir.AluOpType

B, Te, Td, D, ATT = 8, 512, 128, 512, 256
P = 128          # partitions
EC = D // P      # 4 e-chunks
SC = Te // P     # 4 s-chunks
AC = ATT // P    # 2 a-chunks
TB = 8           # decoder steps per tanh buffer
NB = Td // TB    # 16 buffers
WAVE = 32        # softmax wave (rows of scores completed together)
N_CORES = 8


def build_bass() -> bass.Bass:
    nc = bacc.Bacc("TRN2", target_bir_lowering=False, debug=False)

    values_h = nc.declare_dram_parameter("values", [Te, D], F32, isOutput=False)
    query_h = nc.declare_dram_parameter("query", [Td, D], F32, isOutput=False)
    wh_h = nc.declare_dram_parameter("W_h", [D, ATT], F32, isOutput=False)
    ua_h = nc.declare_dram_parameter("U_a", [D, ATT], F32, isOutput=False)
    va_h = nc.declare_dram_parameter("V_a", [1, ATT], F32, isOutput=False)
    c_out_h = nc.declare_dram_parameter("c_out", [Td, D], F32, isOutput=True)
    e_out_h = nc.declare_dram_parameter("e_out", [Td, Te], F32, isOutput=True)

    with tile.TileContext(nc) as tc:
        with (
            tc.tile_pool(name="consts", bufs=1) as consts,
            tc.tile_pool(name="statics", bufs=1) as statics,
            tc.tile_pool(name="pre", bufs=3) as pre_pool,
            tc.tile_pool(name="acts", bufs=3) as act_pool,
            tc.tile_pool(name="scat", bufs=4) as scat_pool,
            tc.tile_pool(name="misc_ps", bufs=2, space="PSUM") as misc_ps,
            tc.tile_pool(name="score_ps", bufs=4, space="PSUM") as score_ps,
        ):
            identity = consts.tile([P, P], F32)
            make_identity(nc, identity)
            identity_bf = consts.tile([P, P], BF16)
            nc.gpsimd.tensor_copy(out=identity_bf, in_=identity)

            # ---------------- load inputs (values first: longest pole) -------
            # values arrives in COLUMN chunks so the transpose -> W_hiT
            # pipeline can run per-e-chunk as data lands.
            values_sb = statics.tile([P, SC, D], F32)    # [s-part, s-chunk, e]
            values_r = values_h[:].rearrange("(c p) e -> p c e", p=P)
            for sc in range(SC):
                # alternate the two HWDGE engines so transfers run in parallel
                eng = nc.scalar if sc % 2 == 0 else nc.sync
                eng.dma_start(out=values_sb[:, sc, :], in_=values_r[:, sc, :])
            wh_sb = statics.tile([P, EC, ATT], F32)      # [e-part, e-chunk, a]
            nc.scalar.dma_start(
                out=wh_sb, in_=wh_h[:].rearrange("(c p) a -> p c a", p=P)
            )
            query_sb = statics.tile([P, D], F32)         # [t, e2]
            nc.sync.dma_start(out=query_sb, in_=query_h[:])
            ua_sb = statics.tile([P, EC, ATT], F32)
            nc.sync.dma_start(
                out=ua_sb, in_=ua_h[:].rearrange("(c p) a -> p c a", p=P)
            )
            v_sb = statics.tile([P, AC], F32)            # V_a chunks on partitions
            nc.sync.dma_start(
                out=v_sb, in_=va_h[:].rearrange("o (c p) -> p (o c)", p=P)
            )
            v_bf = statics.tile([P, AC], BF16)
            nc.vector.tensor_copy(out=v_bf, in_=v_sb)

            # ---------------- valuesT via PE transpose (bf16) ----------------
            # valt_bf[:, ec, :] = values[:, ec-chunk].T  -> [e-part, e-chunk, s]
            # Batched per s-chunk so each batch starts as soon as that values
            # chunk's cast lands; one PSUM-bank drain per batch.
            values_bf = statics.tile([P, SC, D], BF16)
            for sc in range(SC):
                nc.vector.tensor_copy(
                    out=values_bf[:, sc, :], in_=values_sb[:, sc, :]
                )
            valt_bf = statics.tile([P, EC, Te], BF16)
            for sc in range(SC):
                tp_ps = misc_ps.tile([P, EC, P], BF16, tag="ps_misc")
                for ec in range(EC):
                    nc.tensor.transpose(
                        tp_ps[:, ec, :],
                        values_bf[:, sc, ec * P:(ec + 1) * P],
                        identity_bf,
                    )
                nc.vector.tensor_copy(
                    out=valt_bf[:, :, sc * P:(sc + 1) * P], in_=tp_ps
                )

            # casts needed a bit later; behind the valuesT drains on DVE
            wh_bf = statics.tile([P, EC, ATT], BF16)
            nc.vector.tensor_copy(out=wh_bf, in_=wh_sb)
            ua_bf = statics.tile([P, EC, ATT], BF16)
            nc.vector.tensor_copy(out=ua_bf, in_=ua_sb)

            # ---------------- W_hiT = (values @ W_h).T  [a, s]  (bf16) -------
            whT_bf = statics.tile([P, AC, Te], BF16)
            wh_ps = misc_ps.tile([P, AC, Te], F32, tag="ps_wh", bufs=1)
            for ec in range(EC):
                for ai in range(AC):
                    nc.tensor.matmul(
                        wh_ps[:, ai, :],
                        wh_bf[:, ec, ai * P:(ai + 1) * P],   # [e-chunk, a-chunk]
                        valt_bf[:, ec, :],                   # [e-chunk, s]
                        start=(ec == 0),
                        stop=(ec == EC - 1),
                    )
            for ai in range(AC):
                nc.scalar.copy(out=whT_bf[:, ai, :], in_=wh_ps[:, ai, :])

            # ---------------- queryT via PE transpose ------------------------
            qT_bf = statics.tile([P, EC, Td], BF16)      # [e2-part, e2-chunk, t]
            tq_ps = misc_ps.tile([P, EC, P], F32, tag="ps_misc")
            for qc in range(EC):
                nc.tensor.transpose(
                    tq_ps[:, qc, :], query_sb[:, qc * P:(qc + 1) * P], identity
                )
            nc.scalar.copy(out=qT_bf, in_=tq_ps)

            # ---------------- U_sT = (query @ U_a).T  [a, t] -----------------
            usT_sb = statics.tile([P, AC, Td], F32)
            us_ps = misc_ps.tile([P, AC, Td], F32, tag="ps_misc")
            for ai in range(AC):
                for qc in range(EC):
                    nc.tensor.matmul(
                        us_ps[:, ai, :],
                        ua_bf[:, qc, ai * P:(ai + 1) * P],
                        qT_bf[:, qc, :],
                        start=(qc == 0),
                        stop=(qc == EC - 1),
                    )
            nc.scalar.copy(out=usT_sb, in_=us_ps)

            # ---------------- main loop: tanh cube + V reduction -------------
            # Score drains are emitted LAG groups behind their producers so the
            # in-order DVE queue never blocks on a drain whose PE/ACT inputs
            # aren't ready (that would stall the next buffer's pre-adds).
            scores_sb = statics.tile([P, Te], F32)       # [t, s]
            p_sb = statics.tile([P, Te], F32)            # exp(scores - max)
            p_bf = statics.tile([P, Te], BF16)
            negmax = statics.tile([P, 1], F32)
            ssum = statics.tile([P, 1], F32)
            rsum = statics.tile([P, 1], F32)
            e_sb = statics.tile([P, Te], F32)

            LAG = 2
            pending = []
            drained = 0

            def emit_drain():
                nonlocal drained
                sc_ps, g = pending.pop(0)
                scat = scat_pool.tile([P, Te], F32, tag="scat")
                nc.vector.tensor_copy(out=scat, in_=sc_ps)
                # compact partitions {0,32,64,96} -> scores rows 4g..4g+3
                scat_strided = scat[:].rearrange(
                    "(j r) f -> j r f", r=32
                )[:, 0, :]
                nc.sync.dma_start(
                    out=scores_sb[4 * g:4 * g + 4, :], in_=scat_strided
                )
                drained += 1
                if drained % (WAVE // 4) == 0:
                    emit_softmax_wave(drained // (WAVE // 4) - 1)

            def emit_softmax_wave(w):
                # rows 32w..32w+32 of scores are compacted; softmax them.
                rows = slice(WAVE * w, WAVE * (w + 1))
                nc.vector.tensor_reduce(
                    out=negmax[rows], in_=scores_sb[rows, :],
                    axis=mybir.AxisListType.X, op=ALU.max, negate=True,
                )
                nc.scalar.activation(
                    out=p_sb[rows, :], in_=scores_sb[rows, :], func=AF.Exp,
                    bias=negmax[rows], scale=1.0,
                )
                nc.vector.reduce_sum(
                    out=ssum[rows], in_=p_sb[rows, :],
                    axis=mybir.AxisListType.X,
                )
                nc.vector.reciprocal(out=rsum[rows], in_=ssum[rows])
                nc.vector.tensor_scalar_mul(
                    e_sb[rows, :], in0=p_sb[rows, :], scalar1=rsum[rows]
                )
                nc.sync.dma_start(out=e_out_h[rows, :], in_=e_sb[rows, :])
                # bf16 copy of p for the tail's transposes + context matmul
                nc.vector.tensor_copy(out=p_bf[rows, :], in_=p_sb[rows, :])

            if os.environ.get("RECIP_TEST"):
                rt_in = statics.tile([P, 2, Te], F32)
                rt_out = statics.tile([P, 2, Te], F32)
                nc.vector.tensor_scalar(
                    out=rt_in, in0=whT_bf, scalar1=1.5, scalar2=None,
                    op0=ALU.add,
                )
                for _ in range(2):
                    nc.vector.reciprocal(out=rt_out, in_=rt_in)

            # First buffers are small so the first tanh starts earlier; the
            # last is small so the post-loop chain starts on a short tanh.
            sizes = [2, 2, 4] + [TB] * ((Td - 16) // TB) + [4, 4]
            assert sum(sizes) == Td
            slots = {}                               # t -> (act_tile, j)
            t0 = 0
            for tbn in sizes:
                pre = pre_pool.tile([P, TB, AC, Te], BF16, tag="pre")
                for ai in range(AC):
                    for j in range(tbn):
                        t = t0 + j
                        # pre[a, s] = W_hiT[a, s] + U_sT[a, t]
                        nc.vector.tensor_scalar(
                            out=pre[:, j, ai, :],
                            in0=whT_bf[:, ai, :],
                            scalar1=usT_sb[:, ai, t:t + 1],
                            scalar2=None,
                            op0=ALU.add,
                        )
                act = act_pool.tile([P, TB, AC, Te], BF16, tag="act")
                nc.scalar.activation(
                    out=act[:, :tbn], in_=pre[:, :tbn], func=AF.Tanh
                )
                for j in range(tbn):
                    slots[t0 + j] = (act, j)
                t0 += tbn

                while (4 * len(pending) + 4 * drained) + 4 <= t0:
                    g = len(pending) + drained       # next group to emit
                    sc_ps = score_ps.tile([P, Te], F32, tag="score")
                    for j2 in range(4):
                        a_tile, jj = slots.pop(4 * g + j2)
                        for ai in range(AC):
                            # score[t, :] += V[a-chunk] . act[a-chunk, :]
                            nc.tensor.matmul(
                                sc_ps[32 * j2:32 * j2 + 1, :],
                                v_bf[:, ai:ai + 1],
                                a_tile[:, jj, ai, :],
                                start=(ai == 0),
                                stop=(ai == AC - 1),
                                tile_position=(0, 32 * j2),
                            )
                    pending.append((sc_ps, g))
                    if len(pending) > LAG:
                        emit_drain()
            while pending:
                emit_drain()

            # keep the PE busy through the tail softmax so HAM stays at full
            # clock for the pT transposes + context matmuls below
            warm_ps = misc_ps.tile([P, Te], F32, tag="ps_wh", bufs=1)
            for i in range(16):
                nc.tensor.matmul(
                    warm_ps, identity_bf, whT_bf[:, 0, :],
                    start=(i == 0), stop=(i == 15),
                )

            # ---------------- c = (p @ values) * rsum ------------------------
            pT_bf = statics.tile([P, SC, Td], BF16)      # [s-part, s-chunk, t]
            pt_ps = misc_ps.tile([P, SC, P], BF16, tag="ps_misc")
            for sc in range(SC):
                nc.tensor.transpose(
                    pt_ps[:, sc, :], p_bf[:, sc * P:(sc + 1) * P], identity_bf
                )
                nc.vector.tensor_copy(out=pT_bf[:, sc, :], in_=pt_ps[:, sc, :])

            c_ps = misc_ps.tile([P, D], F32, tag="ps_misc")
            for sc in range(SC):
                nc.tensor.matmul(
                    c_ps,
                    pT_bf[:, sc, :],                     # [s-chunk, t]
                    values_bf[:, sc, :],                 # [s-chunk, e]
                    start=(sc == 0),
                    stop=(sc == SC - 1),
                )
            c_sb = statics.tile([P, D], F32)
            nc.scalar.activation(
                out=c_sb, in_=c_ps, func=AF.Copy, scale=rsum
            )
            nc.sync.dma_start(out=c_out_h[:], in_=c_sb)

    nc.compile()
    return nc


_NC_CACHE = None


def _get_nc():
    global _NC_CACHE
    if _NC_CACHE is None:
        _NC_CACHE = build_bass()
    return _NC_CACHE


def run(inputs: dict, trace: bool = False, **kw):
    """Run the SPMD kernel on 8 cores. Returns (BassKernelResults, c, e)."""
    values = np.asarray(inputs["values"], dtype=np.float32)
    query = np.asarray(inputs["query"], dtype=np.float32)
    w_h = np.ascontiguousarray(np.asarray(inputs["W_h"], dtype=np.float32))
    u_a = np.ascontiguousarray(np.asarray(inputs["U_a"], dtype=np.float32))
    v_a = np.ascontiguousarray(np.asarray(inputs["V_a"], dtype=np.float32))

    in_maps = [
        {
            "values": np.ascontiguousarray(values[i]),
            "query": np.ascontiguousarray(query[i]),
            "W_h": w_h,
            "U_a": u_a,
            "V_a": v_a,
        }
        for i in range(N_CORES)
    ]
    res = run_bass_kernel_spmd(
        _get_nc(), in_maps, list(range(N_CORES)), trace=trace, **kw
    )
    c = np.stack([res.results[i]["c_out"] for i in range(N_CORES)])
    e = np.stack([res.results[i]["e_out"] for i in range(N_CORES)])
    return res, c, e


def kernel(**inputs) -> tuple:
    _, c, e = run(inputs)
    return c, e


if __name__ == "__main__":
    rng = np.random.default_rng(0)
    ins = {
        "values": rng.standard_normal((B, Te, D), dtype=np.float32),
        "query": rng.standard_normal((B, Td, D), dtype=np.float32),
        "W_h": rng.uniform(-0.05, 0.05, (D, ATT)).astype(np.float32),
        "U_a": rng.uniform(-0.05, 0.05, (D, ATT)).astype(np.float32),
        "V_a": rng.uniform(-0.05, 0.05, (1, ATT)).astype(np.float32),
    }
    c, e = kernel(**ins)
    print("c", c.shape, c.dtype, "e", e.shape, e.dtype)
